# revision 8
# baseline (speedup 1.0000x reference)
"""Causal multi-head attention (B=2, T=2048, C=1024, H=16) on 8 TRN2 cores.

Sharding: data-parallel over batch (2 groups of 4 cores), tensor-parallel
over heads within a group (4 heads / core). Each core:
  1. computes Q^T, K^T (layout [d, t]) and V (layout [t, d]) for its heads
     from x[b]^T (host-transposed) and its W column slices,
  2. runs causal attention in the S^T = K @ Q^T orientation (softmax sums
     come for free from a ones-column appended to V; softmax max-subtraction
     is skipped -- scores are O(1) here so exp is safe),
  3. AllGathers the per-head attention outputs across the 4 cores of its
     batch group (replica groups [0-3], [4-7]),
  4. computes a 256-column slice of the output projection over the group's
     1024 features.
Host reassembles the 8 [2048, 256] shards into [2, 2048, 1024].

Perf notes vs the first working version:
  - AllGather is group-local (4 cores): half the traffic, and the output
    projection contracts over 1024 rows instead of 2048 (no zero padding).
  - exp runs on k-tile PAIRS ([128, 1024] activations) to halve the
    per-instruction overhead on the Scalar engine (the attention-phase
    bottleneck at full PE speed).
  - softmax normalization uses reciprocal_approx_fast (single DVE op,
    ~5x faster than the bit-exact iterative reciprocal) and multiplies
    straight out of PSUM (no staging copy).
  - causal-mask multiplies only touch the columns that can be masked
    ((dm+1)*128 of 512 per diagonal k-tile).
  - V-projection bias is folded into the output bias host-side
    (bo' = bo + bv @ Wo).
  - weight DMAs ride the Activation engine's HWDGE queue in parallel with
    x^T on the Sync queue; wq/wk/wv are packed into one [C, 768] tensor
    (one DMA per c-tile instead of three).
  - Q projection is emitted c-tile-outer so the PE starts as soon as the
    first x^T tile lands instead of after the whole x^T load.

Matmul operands are bf16 (fp32 PSUM accumulation).
"""

import os
import sys

import numpy as np
import ml_dtypes

for _p in ("/opt/trn_rl_repo",):
    if os.path.isdir(_p) and _p not in sys.path:
        sys.path.insert(0, _p)

import concourse.bacc as bacc
import concourse.mybir as mybir
import concourse.tile as tile
from concourse import bass_utils

B, T, C, H, D = 2, 2048, 1024, 16, 64
NCORES = 8
GP = 4              # cores per batch group
HPC = H // GP       # heads per core = 4
DS = HPC * D        # per-core head-dim slice = 256
NCT = C // 128      # c-tiles = 8
NQC = T // 512      # q-chunks = 4
NKT = T // 128      # k-tiles = 16

F32 = mybir.dt.float32
F32R = mybir.dt.float32r
BF16 = mybir.dt.bfloat16
AF = mybir.ActivationFunctionType
ALU = mybir.AluOpType
NPBF = ml_dtypes.bfloat16

_PROG = None
LAST_RESULTS = None  # BassKernelResults of the most recent run (for test.py)


def _r(ap):
    return ap.bitcast(F32R)


def _normalize(nc, pn, psA, ones_t, attn_sb, op_ps, h, qc):
    """Divide O' rows 0..63 by the row-sums (row 64), write attn."""
    recip = pn.tile([65, 512], F32R, tag="recip")
    with nc.allow_low_precision(
        reason="f32r typing for matmul feed; storage is fp32"
    ):
        nc.vector.reciprocal(recip[64:65, :], op_ps[64:65, :])
    bc_ps = psA.tile([64, 512], F32, tag="sps")
    nc.tensor.matmul(
        bc_ps[:, :],
        _r(ones_t[64:65, :]),
        recip[64:65, :],
        start=True,
        stop=True,
    )
    bc_sb = pn.tile([64, 512], F32, tag="bc")
    nc.vector.tensor_copy(bc_sb[:, :], bc_ps[:, :])
    aslc = attn_sb[(h, qc // 2)][:, 512 * (qc % 2) : 512 * (qc % 2 + 1)]
    nc.vector.tensor_tensor(aslc, op_ps[0:64, :], bc_sb[:, :], ALU.mult)


def _emit(nc, tc, io):
    (xT, wqkv, wo, bq2, bk2, bo_bc, maskd, onesd, onesv, out_shard) = io

    ag_in = [nc.dram_tensor(f"ag_in{i}", [DS, T // 2], BF16) for i in (0, 1)]
    ag_out = [
        nc.dram_tensor(f"ag_out{i}", [GP * DS, T // 2], BF16)
        for i in (0, 1)
    ]

    with (
        tc.tile_pool(name="outer", bufs=1) as po,
    ):
        # ---- persistent tiles; weights ride the scalar HWDGE queue ----
        bq_sb = po.tile([128, 2], F32, tag="bq")
        nc.scalar.dma_start(bq_sb[:, :], bq2[:, :])
        bk_sb = po.tile([128, 2], F32, tag="bk")
        nc.scalar.dma_start(bk_sb[:, :], bk2[:, :])
        ones_t = po.tile([128, 64], F32R, tag="ones")
        nc.scalar.dma_start(ones_t[:, :], onesd[:, :].bitcast(F32R))
        mask_sb = po.tile([128, 4 * 512], BF16, tag="mask")
        attn_sb = {}
        for h in range(HPC):
            for hf in (0, 1):
                attn_sb[(h, hf)] = po.tile(
                    [64, T // 2], BF16, tag=f"attn{h}_{hf}", name=f"attn{h}_{hf}"
                )

        with (
            tc.tile_pool(name="proj", bufs=1) as pp,
            tc.tile_pool(name="work", bufs=12) as pw,
            tc.tile_pool(name="nrm", bufs=2) as pn,
            tc.tile_pool(name="fin", bufs=1) as pf,
            tc.tile_pool(name="ao", bufs=16) as pao,
            tc.tile_pool(name="osb", bufs=3) as posb,
            tc.tile_pool(name="psA", bufs=3, space="PSUM") as psA,
            tc.tile_pool(name="psO", bufs=1, space="PSUM") as psO,
        ):
            # ---- load x^T (sync queue) and packed wqkv (scalar queue),
            # interleaved per c-tile so the Q projection can start while
            # later tiles are still in flight
            xT_sb, w_sb = [], []
            for ci in range(NCT):
                t_ = pp.tile([128, T], BF16, tag=f"xt{ci}", name=f"xt{ci}")
                nc.sync.dma_start(t_[:, :], xT[128 * ci : 128 * (ci + 1), :])
                xT_sb.append(t_)
                t_ = pp.tile([128, 3 * DS], BF16, tag=f"w{ci}", name=f"w{ci}")
                nc.scalar.dma_start(t_[:, :], wqkv[128 * ci : 128 * (ci + 1), :])
                w_sb.append(t_)
            # remaining constants on the scalar queue (needed later)
            vp_sb = pp.tile([128, HPC * NKT * 65], BF16, tag="vp")
            nc.scalar.dma_start(vp_sb[:, :], onesv[:, :])
            nc.scalar.dma_start(mask_sb[:, :], maskd[:, :])
            wo_sb = []
            for ci in range(NCT):
                t_ = pf.tile([128, DS], BF16, tag=f"wo{ci}", name=f"wo{ci}")
                nc.scalar.dma_start(t_[:, :], wo[128 * ci : 128 * (ci + 1), :])
                wo_sb.append(t_)
            bo_sb = pf.tile([128, DS], F32, tag="bo")
            nc.scalar.dma_start(bo_sb[:, :], bo_bc[:, :])

            # ---- Q^T projection, c-tile-outer: the accumulation for all
            # 4 t-chunks of an mt block proceeds as x^T tiles arrive, so
            # the PE starts ~2us into the x^T load. Two [128,1024] PSUM
            # pair-slots hold the 4 chunk accumulators.
            qT_sb = [
                pp.tile([128, T], BF16, tag=f"qT{mt}", name=f"qT{mt}")
                for mt in range(2)
            ]
            kT_sb = [
                pp.tile([128, T], BF16, tag=f"kT{mt}", name=f"kT{mt}")
                for mt in range(2)
            ]
            for mt in range(2):
                slots = [
                    psA.tile([128, 1024], F32, tag="sps", name=f"qacc{mt}_{i}")
                    for i in range(2)
                ]
                for ci in range(NCT):
                    for tch in range(NQC):
                        nc.tensor.matmul(
                            slots[tch // 2][:, 512 * (tch % 2) : 512 * (tch % 2 + 1)],
                            w_sb[ci][:, 128 * mt : 128 * (mt + 1)],
                            xT_sb[ci][:, 512 * tch : 512 * (tch + 1)],
                            start=(ci == 0),
                            stop=(ci == NCT - 1),
                        )
                for tch in range(NQC):
                    nc.vector.tensor_scalar_add(
                        qT_sb[mt][:, 512 * tch : 512 * (tch + 1)],
                        slots[tch // 2][:, 512 * (tch % 2) : 512 * (tch % 2 + 1)],
                        bq_sb[:, mt : mt + 1],
                    )

            # ---- K^T projection (t-chunk-outer; DMAs done by now) -------
            for mt in range(2):
                for tch in range(NQC):
                    ps = psA.tile([128, 1024], F32, tag="sps")
                    for ci in range(NCT):
                        nc.tensor.matmul(
                            ps[:, 0:512],
                            w_sb[ci][:, DS + 128 * mt : DS + 128 * (mt + 1)],
                            xT_sb[ci][:, 512 * tch : 512 * (tch + 1)],
                            start=(ci == 0),
                            stop=(ci == NCT - 1),
                        )
                    nc.vector.tensor_scalar_add(
                        kT_sb[mt][:, 512 * tch : 512 * (tch + 1)],
                        ps[:, 0:512],
                        bk_sb[:, mt : mt + 1],
                    )

            # ---- V projection: out [t, d] + ones column ----------------
            for tt in range(NKT):
                ps = psA.tile([128, 1024], F32, tag="sps")
                for ci in range(NCT):
                    nc.tensor.matmul(
                        ps[:, 0:DS],
                        xT_sb[ci][:, 128 * tt : 128 * (tt + 1)],
                        w_sb[ci][:, 2 * DS : 3 * DS],
                        start=(ci == 0),
                        stop=(ci == NCT - 1),
                    )
                for h in range(HPC):
                    nc.vector.tensor_copy(
                        vp_sb[:, 1040 * h + 65 * tt : 1040 * h + 65 * tt + 64],
                        ps[:, 64 * h : 64 * (h + 1)],
                    )

            # ---- causal attention (S^T orientation) --------------------
            # k-tiles are processed in PAIRS: both S matmuls of a pair land
            # in one [128, 1024] PSUM tile and a single exp covers both,
            # halving Scalar-engine instruction overhead. Head pairs share
            # each step; PV accumulations trail the S/exp front by 2 pairs.
            for qc in range(NQC):
                nkp = 2 * qc + 2  # k-tile pairs this q-chunk
                nkt = 2 * nkp
                for hp in (0, 2):
                    ops = {}
                    for h in (hp, hp + 1):
                        ops[h] = psO.tile(
                            [65, 512], F32, tag=f"ops{h % 2}", name=f"op_q{qc}h{h}"
                        )
                    pTs = {}
                    for kp in range(nkp + 2):
                        if kp < nkp:
                            for h in (hp, hp + 1):
                                mt, pof = h // 2, 64 * (h % 2)
                                qs = qT_sb[mt][pof : pof + 64, 512 * qc : 512 * (qc + 1)]
                                st = psA.tile([128, 1024], F32, tag="sps")
                                for j in (0, 1):
                                    kt = 2 * kp + j
                                    nc.tensor.matmul(
                                        st[:, 512 * j : 512 * (j + 1)],
                                        kT_sb[mt][pof : pof + 64, 128 * kt : 128 * (kt + 1)],
                                        qs,
                                        start=True,
                                        stop=True,
                                    )
                                pT = pw.tile([128, 1024], BF16, tag="pT")
                                nc.scalar.activation(
                                    pT[:, :], st[:, :], AF.Exp, scale=0.125
                                )
                                for j in (0, 1):
                                    dm = 2 * kp + j - 4 * qc
                                    if dm >= 0:  # diagonal: mask cols that can hide
                                        mw = (dm + 1) * 128
                                        nc.vector.tensor_tensor(
                                            pT[:, 512 * j : 512 * j + mw],
                                            pT[:, 512 * j : 512 * j + mw],
                                            mask_sb[:, 512 * dm : 512 * dm + mw],
                                            ALU.mult,
                                        )
                                pTs[(h, kp)] = pT
                        kv = kp - 2
                        if kv >= 0:
                            for h in (hp, hp + 1):
                                pT = pTs.pop((h, kv))
                                for j in (0, 1):
                                    k2 = 2 * kv + j
                                    nc.tensor.matmul(
                                        ops[h][:, :],
                                        vp_sb[:, 1040 * h + 65 * k2 : 1040 * h + 65 * k2 + 65],
                                        pT[:, 512 * j : 512 * (j + 1)],
                                        start=(k2 == 0),
                                        stop=(k2 == nkt - 1),
                                    )
                    for h in (hp, hp + 1):
                        _normalize(nc, pn, psA, ones_t, attn_sb, ops[h], h, qc)

            # ---- group AllGather per q-half (the first can ship while
            # the second half of attention is still computing) ------------
            groups = [[0, 1, 2, 3], [4, 5, 6, 7]]
            for hf in (0, 1):
                for h in range(HPC):
                    nc.sync.dma_start(
                        ag_in[hf][64 * h : 64 * (h + 1), :],
                        attn_sb[(h, hf)][:, :],
                    )
                nc.gpsimd.collective_compute(
                    "AllGather",
                    ALU.bypass,
                    replica_groups=groups,
                    ins=[ag_in[hf][:, :]],
                    outs=[ag_out[hf][:, :]],
                )

            # ---- output projection: full T, 256-column slice of Wo over
            # the group's 1024 gathered features ---------------------------
            for hf in (0, 1):
                for tg2 in (0, 1):
                    tg = 2 * hf + tg2
                    ao_t = []
                    for ci in range(NCT):
                        t_ = pao.tile([128, 512], BF16, tag="ao")
                        dq = nc.scalar if hf == 1 else nc.sync
                        dq.dma_start(
                            t_[:, :],
                            ag_out[hf][
                                128 * ci : 128 * (ci + 1), 512 * tg2 : 512 * (tg2 + 1)
                            ],
                        )
                        ao_t.append(t_)
                    for tj in range(4):
                        tt = 4 * tg + tj
                        ps = psO.tile(
                            [128, DS], F32, tag=f"ops{tt % 2}", name=f"out_ps{tt}"
                        )
                        for ci in range(NCT):
                            nc.tensor.matmul(
                                ps[:, :],
                                ao_t[ci][:, 128 * tj : 128 * (tj + 1)],
                                wo_sb[ci][:, :],
                                start=(ci == 0),
                                stop=(ci == NCT - 1),
                            )
                        osb = posb.tile([128, DS], F32, tag="osb")
                        nc.vector.tensor_tensor(
                            osb[:, :], ps[:, :], bo_sb[:, :], ALU.add
                        )
                        nc.sync.dma_start(
                            out_shard[128 * tt : 128 * (tt + 1), :], osb[:, :]
                        )


def _build_program():
    nc = bacc.Bacc(
        "TRN2",
        target_bir_lowering=False,
        debug=False,
        num_devices=NCORES,
    )
    xT = nc.dram_tensor("xT", [C, T], BF16, kind="ExternalInput")
    wqkv = nc.dram_tensor("wqkv", [C, 3 * DS], BF16, kind="ExternalInput")
    wo = nc.dram_tensor("wo", [C, DS], BF16, kind="ExternalInput")
    bq2 = nc.dram_tensor("bq2", [128, 2], F32, kind="ExternalInput")
    bk2 = nc.dram_tensor("bk2", [128, 2], F32, kind="ExternalInput")
    bo_bc = nc.dram_tensor("bo_bc", [128, DS], F32, kind="ExternalInput")
    maskd = nc.dram_tensor("maskd", [128, 4 * 512], BF16, kind="ExternalInput")
    onesd = nc.dram_tensor("onesd", [128, 64], F32, kind="ExternalInput")
    onesv = nc.dram_tensor(
        "onesv", [128, HPC * NKT * 65], BF16, kind="ExternalInput"
    )
    out_shard = nc.dram_tensor("out_shard", [T, DS], F32, kind="ExternalOutput")
    io = (xT, wqkv, wo, bq2, bk2, bo_bc, maskd, onesd, onesv, out_shard)
    with tile.TileContext(nc) as tc:
        _emit(nc, tc, io)
    nc.compile()
    return nc


def _make_mask():
    # multiplicative causal mask blocks for the 4 diagonal positions:
    # 1 where k is visible (128*m + k_local <= q_local), 0 otherwise
    k = np.arange(128, dtype=np.int64)[:, None]
    q = np.arange(512, dtype=np.int64)[None, :]
    mask = np.zeros((128, 4 * 512), np.float32)
    for m in range(4):
        mask[:, 512 * m : 512 * (m + 1)] = (128 * m + k <= q).astype(np.float32)
    return mask.astype(NPBF)


def _make_in_maps(x, Wq, bq, Wk, bk, Wv, bv, Wo, bo):
    mask = _make_mask()
    in_maps = []
    for c in range(NCORES):
        b, g = c // GP, c % GP
        hs = slice(DS * g, DS * (g + 1))
        wqkv = np.concatenate([Wq[:, hs], Wk[:, hs], Wv[:, hs]], axis=1)
        bo_eff = bo[hs] + bv @ Wo[:, hs]  # V bias folded through Wo
        in_maps.append(
            {
                "xT": np.ascontiguousarray(x[b].T).astype(NPBF),
                "wqkv": np.ascontiguousarray(wqkv).astype(NPBF),
                "wo": np.ascontiguousarray(Wo[:, hs]).astype(NPBF),
                "bq2": np.ascontiguousarray(bq[hs].reshape(2, 128).T),
                "bk2": np.ascontiguousarray(bk[hs].reshape(2, 128).T),
                "bo_bc": np.tile(bo_eff[None, :], (128, 1)).astype(np.float32),
                "maskd": mask,
                "onesd": np.ones((128, 64), np.float32),
                "onesv": np.ones((128, HPC * NKT * 65), NPBF),
            }
        )
    return in_maps


def kernel(x, Wq, bq, Wk, bk, Wv, bv, Wo, bo, _trace=False, _trace_cores=None):
    global _PROG, LAST_RESULTS
    x = np.asarray(x, np.float32)
    Wq, bq = np.asarray(Wq, np.float32), np.asarray(bq, np.float32)
    Wk, bk = np.asarray(Wk, np.float32), np.asarray(bk, np.float32)
    Wv, bv = np.asarray(Wv, np.float32), np.asarray(bv, np.float32)
    Wo, bo = np.asarray(Wo, np.float32), np.asarray(bo, np.float32)

    if _PROG is None:
        _PROG = _build_program()
    nc = _PROG

    in_maps = _make_in_maps(x, Wq, bq, Wk, bk, Wv, bv, Wo, bo)

    kw = {}
    if _trace:
        kw["trace"] = True
        if _trace_cores is not None:
            kw["trace_cores"] = _trace_cores
    res = bass_utils.run_bass_kernel_spmd(nc, in_maps, list(range(NCORES)), **kw)
    LAST_RESULTS = res

    out = np.empty((B, T, C), np.float32)
    for c in range(NCORES):
        b, g = c // GP, c % GP
        out[b, :, DS * g : DS * (g + 1)] = res.results[c]["out_shard"]
    return out
